# revision 10
# baseline (speedup 1.0000x reference)
"""VQ-codebook 3x3 conv (nn_CConv) on 8 Trainium2 NeuronCores.

Sharding: data-parallel over the batch (16 images -> 2 per core); the small
codebook-derived weights / scales / bias are replicated to every core.
Host-side work is layout only: batch split, reshape/transpose of the index
and scale matrices, and the codebook row gather (pure indexing, no
arithmetic).

Per-core device program (one NEFF, SPMD over 8 cores):
  - weight build (on device): fp16 round-trip of scales (dequant emulation),
    multiply by cut, broadcast-multiply onto the gathered codebook rows;
    per-tap multiplies so tap 0's stationary block is ready early.
  - conv: images are zero-padded into 113-stride rows (consecutive rows
    share one zero border column: position j*113 is both row j-1's right
    pad and row j's left pad), so each output row costs 113 PE columns
    (1 junk) instead of 114 (2 junk).  The 3x3 conv is 9 accumulating PE
    matmuls over shifted views of the flat padded image, fp16 in / fp32
    PSUM accumulate.  PSUM tiles are 4-output-row aligned (452 = 4*113
    cols); the vector-engine bias-add evacuation drops the per-row junk
    column so the SBUF output slab is fully contiguous and the output DMA
    is a single linear run per partition.
  - images are processed in row-slabs sized progressively (2-row bootstrap
    slabs first, tapering small again at the end): tiny slabs at the two
    ends shorten the serial prologue and the final-DMA tail, while each
    slab's compute covers the next slab's load+cast.
  - DMA queues run at ~1/3 bandwidth for the first ~6us (cold clock gate)
    and DMA-trigger instructions cost ~700ns each, so the prologue is
    planned around bytes and trigger slots: weights travel as two packed
    f16 tensors (one per HWDGE queue, one trigger each); input loads ride
    the sync queue; casts f32->f16 and the weight build share the vector
    engine, interleaved in need-order; PSUM evacuation (fused bias add)
    rides the scalar engine; output DMAs ride the scalar queue so they
    never head-of-line-block input loads.
  - dummy warm-up matmuls keep the PE busy from the kernel-body start so
    the HAM clock gate reaches 2.4 GHz with no idle-demote, ending right
    when the first real slab + weights are ready.
"""
import sys
import types
from contextlib import ExitStack

import numpy as np

import concourse.tile as tile
from concourse import bacc, mybir


def _ensure_axon_hooks_module():
    """This image's antenv package lacks axon_hooks; bass_utils imports it
    when tracing is requested (e.g. BASS_TRACE=1). Provide a no-op shim."""
    try:
        import antenv

        if "antenv.axon_hooks" not in sys.modules and not hasattr(
            antenv, "axon_hooks"
        ):
            mod = types.ModuleType("antenv.axon_hooks")
            holder = [None]
            mod.set_axon_ntff_profile_hook = lambda h: holder.__setitem__(0, h)
            mod.get_axon_ntff_profile_hook = lambda: holder[0]
            antenv.axon_hooks = mod
            sys.modules["antenv.axon_hooks"] = mod
    except Exception:
        pass


_ensure_axon_hooks_module()

from concourse import bass_utils  # noqa: E402

P = 128
H = W = 112
WP = 113  # padded row stride (shared border zeros)
IMGS = 2
N_CORES = 8
TROWS = 4  # output rows per PSUM tile
TCOLS = TROWS * WP  # 452

f32 = mybir.dt.float32
f16 = mybir.dt.float16

SLAB_PLAN = {0: [2, 2, 4, 8, 12, 16, 24, 28, 16], 1: [28, 28, 24, 16, 8, 4, 2, 2]}
MAX_SO = 28
WARM_MMS = 11

_CACHE = {}


def _build():
    nc = bacc.Bacc("TRN2", target_bir_lowering=False, debug=False)

    x_t = nc.dram_tensor("x", [IMGS, P, H, W], f32, kind="ExternalInput")
    # wpackA: scalesT f16 | cutT f16 | bias f32-as-2xf16 | wraw taps 0-2
    wpackA_t = nc.dram_tensor("wpackA", [P, 2 * P + 2 + 3 * P], f16,
                              kind="ExternalInput")
    # wpackB: wraw taps 3-8
    wpackB_t = nc.dram_tensor("wpackB", [P, 6 * P], f16, kind="ExternalInput")
    out_t = nc.dram_tensor("out", [IMGS, P, H, W], f32, kind="ExternalOutput")

    with tile.TileContext(nc) as tc, ExitStack() as ctx:
        wb = ctx.enter_context(tc.tile_pool(name="wb", bufs=1))
        xp = ctx.enter_context(tc.tile_pool(name="xp", bufs=3))
        op = ctx.enter_context(tc.tile_pool(name="op", bufs=3))
        ps = ctx.enter_context(tc.tile_pool(name="ps", bufs=8, space="PSUM"))
        xs = ctx.enter_context(tc.tile_pool(name="xs", bufs=3))

        max_xpad_len = (MAX_SO + 2) * WP + 2

        def stage_slab(img, h0, so, queue=None):
            """Issue memsets + staged load + vector cast for one slab; return
            the padded f16 tile."""
            slab_in = so + 2
            xlen = slab_in * WP
            xpad = xp.tile([P, max_xpad_len], f16, tag="xpad")
            xpad3 = xpad[:, :xlen].rearrange("p (r c) -> p r c", c=WP)
            # shared border zeros: cols {0,1} of every 113-stride row
            # (col 1 is overwritten by the cast for real rows) + 2 tail
            # elements (tap over-read feeds junk outputs only)
            nc.gpsimd.memset(xpad3[:, :, 0:2], 0.0)
            nc.gpsimd.memset(xpad[:, xlen:xlen + 2], 0.0)
            if h0 == 0:
                nc.gpsimd.memset(xpad[:, 0:WP], 0.0)
            elif h0 + so == H:
                nc.gpsimd.memset(xpad[:, (slab_in - 1) * WP:xlen], 0.0)
            # interior rows: f32 staged load, vector-engine cast to f16
            r_lo = max(0, h0 - 1)
            r_hi = min(H, h0 + so + 1)
            j0 = r_lo - (h0 - 1)
            nrows = r_hi - r_lo
            stage = xs.tile([P, (MAX_SO + 2) * W], f32, tag="xstage")
            (queue or nc.sync).dma_start(
                stage[:, :nrows * W], x_t.ap()[img, :, r_lo:r_hi, :]
            )
            nc.vector.tensor_copy(
                xpad3[:, j0:j0 + nrows, 1:1 + W],
                stage[:, :nrows * W].rearrange("p (r c) -> p r c", c=W),
            )
            return xpad

        def compute_slab(img, h0, so, xpad):
            """Issue matmuls + fused-bias evacuation + output DMA for a
            staged slab."""
            oslab = op.tile([P, MAX_SO * W], f32, tag="oslab")
            q0 = 0
            trows_list = ([2] if so % 4 else []) + [TROWS] * (so // TROWS)
            for trows in trows_list:
                ncols = trows * WP
                pst = ps.tile([P, 512], f32, tag="pst")
                for k in range(9):
                    dh, dw = divmod(k, 3)
                    off = q0 * WP + dh * WP + dw
                    nc.tensor.matmul(
                        pst[:, :ncols],
                        w_mm[:, k * P:(k + 1) * P],
                        xpad[:, off:off + ncols],
                        start=(k == 0),
                        stop=(k == 8),
                    )
                # evacuate with fused bias add (scalar engine), dropping the
                # per-row junk column so oslab is contiguous
                nc.scalar.add(
                    oslab[:, q0 * W:(q0 + trows) * W].rearrange(
                        "p (r c) -> p r c", c=W
                    ),
                    pst[:, :ncols].rearrange("p (r c) -> p r c", c=WP)[
                        :, :, 0:W
                    ],
                    bias_s,
                )
                q0 += trows
            # fully linear per-partition output DMA
            nc.scalar.dma_start(
                out_t.ap()[img, :, h0:h0 + so, :], oslab[:, :so * W]
            )

        # ---- prologue: packed weight loads, one trigger per queue ----
        pkA = wb.tile([P, 2 * P + 2 + 3 * P], f16, tag="pkA")
        nc.sync.dma_start(pkA[:], wpackA_t.ap())
        pkB = wb.tile([P, 6 * P], f16, tag="pkB")
        nc.scalar.dma_start(pkB[:], wpackB_t.ap())
        bias_s = pkA[:, 2 * P:2 * P + 2].bitcast(f32)

        # first two bootstrap slabs' input loads head the sync queue
        so0 = SLAB_PLAN[0][0]
        so1 = SLAB_PLAN[0][1]

        # PE warmup: keeps PE busy from the kernel-body start so the HAM
        # clock gate promotes to 2.4 GHz before the first real matmul
        wrm = wb.tile([P, 512], f16, tag="warm")
        nc.gpsimd.memset(wrm[:], 0.0)
        pw = ps.tile([P, 512], f32, tag="pst")
        for _ in range(WARM_MMS):
            nc.tensor.matmul(pw[:], wrm[:, :P], wrm[:], start=True, stop=True)

        # ---- weight build, interleaved with bootstrap casts in need-order
        # (all on the vector engine: no DMA triggers or table loads there) ----
        scc16 = wb.tile([P, P], f16, tag="scc16")
        nc.vector.tensor_tensor(
            out=scc16[:], in0=pkA[:, 0:P], in1=pkA[:, P:2 * P],
            op=mybir.AluOpType.mult,
        )
        w_mm = wb.tile([P, 9 * P], f16, tag="w_mm")

        def build_tap(k):
            src = (pkA[:, 2 * P + 2 + k * P:2 * P + 2 + (k + 1) * P]
                   if k < 3 else pkB[:, (k - 3) * P:(k - 2) * P])
            nc.vector.tensor_tensor(
                out=w_mm[:, k * P:(k + 1) * P], in0=src, in1=scc16[:],
                op=mybir.AluOpType.mult,
            )

        for k in range(3):
            build_tap(k)
        xpad0 = stage_slab(0, 0, so0)
        for k in range(3, 6):
            build_tap(k)
        xpad1 = stage_slab(0, so0, so1, queue=nc.scalar)
        for k in range(6, 9):
            build_tap(k)

        # ---- conv slabs ----
        compute_slab(0, 0, so0, xpad0)
        compute_slab(0, so0, so1, xpad1)
        h0 = so0 + so1
        for so in SLAB_PLAN[0][2:]:
            xpad = stage_slab(0, h0, so)
            compute_slab(0, h0, so, xpad)
            h0 += so
        h0 = 0
        for so in SLAB_PLAN[1]:
            xpad = stage_slab(1, h0, so)
            compute_slab(1, h0, so, xpad)
            h0 += so

    nc.compile()
    return nc


def _make_in_maps(inputs):
    x = np.ascontiguousarray(np.asarray(inputs["x"], dtype=np.float32))
    cent = np.asarray(inputs["centroids"], dtype=np.float32).reshape(512, 9)
    idxT = np.asarray(inputs["idx"]).reshape(P, P).T          # [i, o]
    scalesT = np.asarray(inputs["scales"], dtype=np.float32).reshape(P, P).T
    cutT = np.asarray(inputs["cut"], dtype=np.float32).reshape(P, P).T
    bias16 = (
        np.asarray(inputs["bias"], dtype=np.float32)
        .reshape(P, 1).copy().view(np.float16)
    )  # f32 bits carried in 2 f16 lanes, bitcast back on device
    # codebook gather, k-major: wraw[i, k*128+o] = cent[idx[o,i]][k], f16
    wraw = cent[idxT].transpose(0, 2, 1).reshape(P, 9 * P).astype(np.float16)
    wpackA = np.ascontiguousarray(np.concatenate(
        [scalesT.astype(np.float16), cutT.astype(np.float16), bias16,
         wraw[:, 0:3 * P]], axis=1))
    wpackB = np.ascontiguousarray(wraw[:, 3 * P:9 * P])

    base = {"wpackA": wpackA, "wpackB": wpackB}
    maps = []
    for c in range(N_CORES):
        m = dict(base)
        m["x"] = np.ascontiguousarray(x[IMGS * c:IMGS * (c + 1)])
        maps.append(m)
    return maps


def _get_nc():
    if "nc" not in _CACHE:
        _CACHE["nc"] = _build()
    return _CACHE["nc"]


def _run(inputs, trace=False):
    nc = _get_nc()
    in_maps = _make_in_maps(inputs)
    res = bass_utils.run_bass_kernel_spmd(
        nc, in_maps, core_ids=list(range(N_CORES)), trace=trace
    )
    out = np.concatenate([res.results[c]["out"] for c in range(N_CORES)], axis=0)
    return out, res


def kernel(**inputs) -> np.ndarray:
    out, _ = _run(inputs, trace=False)
    return out


# revision 11
# speedup vs baseline: 1.0142x; 1.0142x over previous
"""VQ-codebook 3x3 conv (nn_CConv) on 8 Trainium2 NeuronCores.

Sharding: data-parallel over the batch (16 images -> 2 per core); the small
codebook-derived weights / scales / bias are replicated to every core.
Host-side work is layout only: batch split, reshape/transpose of the index
and scale matrices, and the codebook row gather (pure indexing, no
arithmetic).

Per-core device program (one NEFF, SPMD over 8 cores):
  - weight build (on device): fp16 round-trip of scales (dequant emulation),
    multiply by cut, broadcast-multiply onto the gathered codebook rows;
    per-tap multiplies so tap 0's stationary block is ready early.
  - conv: images are zero-padded into 113-stride rows (consecutive rows
    share one zero border column: position j*113 is both row j-1's right
    pad and row j's left pad), so each output row costs 113 PE columns
    (1 junk) instead of 114 (2 junk).  The 3x3 conv is 9 accumulating PE
    matmuls over shifted views of the flat padded image, fp16 in / fp32
    PSUM accumulate.  PSUM tiles are 4-output-row aligned (452 = 4*113
    cols); the vector-engine bias-add evacuation drops the per-row junk
    column so the SBUF output slab is fully contiguous and the output DMA
    is a single linear run per partition.
  - images are processed in row-slabs sized progressively (2-row bootstrap
    slabs first, tapering small again at the end): tiny slabs at the two
    ends shorten the serial prologue and the final-DMA tail, while each
    slab's compute covers the next slab's load+cast.
  - DMA queues run at ~1/3 bandwidth for the first ~6us (cold clock gate)
    and DMA-trigger instructions cost ~700ns each, so the prologue is
    planned around bytes and trigger slots: weights travel as two packed
    f16 tensors (one per HWDGE queue, one trigger each); input loads ride
    the sync queue; casts f32->f16 and the weight build share the vector
    engine, interleaved in need-order; PSUM evacuation (fused bias add)
    rides the scalar engine; output DMAs ride the scalar queue so they
    never head-of-line-block input loads.
  - dummy warm-up matmuls keep the PE busy from the kernel-body start so
    the HAM clock gate reaches 2.4 GHz with no idle-demote, ending right
    when the first real slab + weights are ready.
"""
import sys
import types
from contextlib import ExitStack

import numpy as np

import concourse.tile as tile
from concourse import bacc, mybir


def _ensure_axon_hooks_module():
    """This image's antenv package lacks axon_hooks; bass_utils imports it
    when tracing is requested (e.g. BASS_TRACE=1). Provide a no-op shim."""
    try:
        import antenv

        if "antenv.axon_hooks" not in sys.modules and not hasattr(
            antenv, "axon_hooks"
        ):
            mod = types.ModuleType("antenv.axon_hooks")
            holder = [None]
            mod.set_axon_ntff_profile_hook = lambda h: holder.__setitem__(0, h)
            mod.get_axon_ntff_profile_hook = lambda: holder[0]
            antenv.axon_hooks = mod
            sys.modules["antenv.axon_hooks"] = mod
    except Exception:
        pass


_ensure_axon_hooks_module()

from concourse import bass_utils  # noqa: E402

P = 128
H = W = 112
WP = 113  # padded row stride (shared border zeros)
IMGS = 2
N_CORES = 8
TROWS = 4  # output rows per PSUM tile
TCOLS = TROWS * WP  # 452

f32 = mybir.dt.float32
f16 = mybir.dt.float16

SLAB_PLAN = {0: [2, 2, 4, 8, 12, 16, 24, 28, 16], 1: [28, 28, 24, 16, 8, 4, 2, 2]}
MAX_SO = 28
WARM_MMS = 11

_CACHE = {}


def _build():
    nc = bacc.Bacc("TRN2", target_bir_lowering=False, debug=False)

    x_t = nc.dram_tensor("x", [IMGS, P, H, W], f32, kind="ExternalInput")
    # wpackA: scalesT f16 | cutT f16 | bias f32-as-2xf16 | wraw taps 0-2
    wpackA_t = nc.dram_tensor("wpackA", [P, 2 * P + 2 + 3 * P], f16,
                              kind="ExternalInput")
    # wpackB: wraw taps 3-8
    wpackB_t = nc.dram_tensor("wpackB", [P, 6 * P], f16, kind="ExternalInput")
    out_t = nc.dram_tensor("out", [IMGS, P, H, W], f32, kind="ExternalOutput")

    with tile.TileContext(nc) as tc, ExitStack() as ctx:
        wb = ctx.enter_context(tc.tile_pool(name="wb", bufs=1))
        xp = ctx.enter_context(tc.tile_pool(name="xp", bufs=3))
        op = ctx.enter_context(tc.tile_pool(name="op", bufs=3))
        ps = ctx.enter_context(tc.tile_pool(name="ps", bufs=7, space="PSUM"))
        xs = ctx.enter_context(tc.tile_pool(name="xs", bufs=3))

        max_xpad_len = (MAX_SO + 2) * WP + 2

        def stage_slab(img, h0, so, queue=None):
            """Issue memsets + staged load + vector cast for one slab; return
            the padded f16 tile."""
            slab_in = so + 2
            xlen = slab_in * WP
            xpad = xp.tile([P, max_xpad_len], f16, tag="xpad")
            xpad3 = xpad[:, :xlen].rearrange("p (r c) -> p r c", c=WP)
            # shared border zeros: cols {0,1} of every 113-stride row
            # (col 1 is overwritten by the cast for real rows) + 2 tail
            # elements (tap over-read feeds junk outputs only)
            nc.gpsimd.memset(xpad3[:, :, 0:2], 0.0)
            nc.gpsimd.memset(xpad[:, xlen:xlen + 2], 0.0)
            if h0 == 0:
                nc.gpsimd.memset(xpad[:, 0:WP], 0.0)
            elif h0 + so == H:
                nc.gpsimd.memset(xpad[:, (slab_in - 1) * WP:xlen], 0.0)
            # interior rows: f32 staged load, vector-engine cast to f16
            r_lo = max(0, h0 - 1)
            r_hi = min(H, h0 + so + 1)
            j0 = r_lo - (h0 - 1)
            nrows = r_hi - r_lo
            stage = xs.tile([P, (MAX_SO + 2) * W], f32, tag="xstage")
            (queue or nc.sync).dma_start(
                stage[:, :nrows * W], x_t.ap()[img, :, r_lo:r_hi, :]
            )
            nc.vector.tensor_copy(
                xpad3[:, j0:j0 + nrows, 1:1 + W],
                stage[:, :nrows * W].rearrange("p (r c) -> p r c", c=W),
            )
            return xpad

        def compute_slab(img, h0, so, xpad, out_queue=None):
            """Issue matmuls + fused-bias evacuation + output DMA for a
            staged slab."""
            oslab = op.tile([P, MAX_SO * W], f32, tag="oslab")
            q0 = 0
            trows_list = ([2] if so % 4 else []) + [TROWS] * (so // TROWS)
            for trows in trows_list:
                ncols = trows * WP
                pst = ps.tile([P, 512], f32, tag="pst")
                for k in range(9):
                    dh, dw = divmod(k, 3)
                    off = q0 * WP + dh * WP + dw
                    nc.tensor.matmul(
                        pst[:, :ncols],
                        w_mm[:, k * P:(k + 1) * P],
                        xpad[:, off:off + ncols],
                        start=(k == 0),
                        stop=(k == 8),
                    )
                # evacuate with fused bias add (scalar engine), dropping the
                # per-row junk column so oslab is contiguous
                nc.scalar.add(
                    oslab[:, q0 * W:(q0 + trows) * W].rearrange(
                        "p (r c) -> p r c", c=W
                    ),
                    pst[:, :ncols].rearrange("p (r c) -> p r c", c=WP)[
                        :, :, 0:W
                    ],
                    bias_s,
                )
                q0 += trows
            # fully linear per-partition output DMA
            (out_queue or nc.scalar).dma_start(
                out_t.ap()[img, :, h0:h0 + so, :], oslab[:, :so * W]
            )

        # ---- prologue: packed weight loads, one trigger per queue ----
        pkA = wb.tile([P, 2 * P + 2 + 3 * P], f16, tag="pkA")
        nc.sync.dma_start(pkA[:], wpackA_t.ap())
        pkB = wb.tile([P, 6 * P], f16, tag="pkB")
        nc.scalar.dma_start(pkB[:], wpackB_t.ap())
        bias_s = pkA[:, 2 * P:2 * P + 2].bitcast(f32)

        # first two bootstrap slabs' input loads head the sync queue
        so0 = SLAB_PLAN[0][0]
        so1 = SLAB_PLAN[0][1]

        # PE warmup: keeps PE busy from the kernel-body start so the HAM
        # clock gate promotes to 2.4 GHz before the first real matmul
        wrm = wb.tile([P, 512], f16, tag="warm")
        nc.gpsimd.memset(wrm[:], 0.0)
        pw = ps.tile([P, 512], f32, tag="pst")
        for _ in range(WARM_MMS):
            nc.tensor.matmul(pw[:], wrm[:, :P], wrm[:], start=True, stop=True)

        # ---- weight build, interleaved with bootstrap casts in need-order
        # (all on the vector engine: no DMA triggers or table loads there) ----
        scc16 = wb.tile([P, P], f16, tag="scc16")
        nc.vector.tensor_tensor(
            out=scc16[:], in0=pkA[:, 0:P], in1=pkA[:, P:2 * P],
            op=mybir.AluOpType.mult,
        )
        w_mm = wb.tile([P, 9 * P], f16, tag="w_mm")

        def build_tap(k):
            src = (pkA[:, 2 * P + 2 + k * P:2 * P + 2 + (k + 1) * P]
                   if k < 3 else pkB[:, (k - 3) * P:(k - 2) * P])
            nc.vector.tensor_tensor(
                out=w_mm[:, k * P:(k + 1) * P], in0=src, in1=scc16[:],
                op=mybir.AluOpType.mult,
            )

        for k in range(3):
            build_tap(k)
        xpad0 = stage_slab(0, 0, so0)
        for k in range(3, 6):
            build_tap(k)
        xpad1 = stage_slab(0, so0, so1, queue=nc.scalar)
        for k in range(6, 9):
            build_tap(k)

        # ---- conv slabs ----
        compute_slab(0, 0, so0, xpad0)
        compute_slab(0, so0, so1, xpad1)
        h0 = so0 + so1
        for so in SLAB_PLAN[0][2:]:
            xpad = stage_slab(0, h0, so)
            compute_slab(0, h0, so, xpad)
            h0 += so
        h0 = 0
        n1 = len(SLAB_PLAN[1])
        for i, so in enumerate(SLAB_PLAN[1]):
            xpad = stage_slab(1, h0, so)
            # the tail output DMAs ride the (idle, warm) sync queue so the
            # epilogue barrier isn't stuck behind queued scalar-side outputs
            oq = nc.sync if i >= n1 - 2 else None
            compute_slab(1, h0, so, xpad, out_queue=oq)
            h0 += so

    nc.compile()
    return nc


def _make_in_maps(inputs):
    x = np.ascontiguousarray(np.asarray(inputs["x"], dtype=np.float32))
    cent = np.asarray(inputs["centroids"], dtype=np.float32).reshape(512, 9)
    idxT = np.asarray(inputs["idx"]).reshape(P, P).T          # [i, o]
    scalesT = np.asarray(inputs["scales"], dtype=np.float32).reshape(P, P).T
    cutT = np.asarray(inputs["cut"], dtype=np.float32).reshape(P, P).T
    bias16 = (
        np.asarray(inputs["bias"], dtype=np.float32)
        .reshape(P, 1).copy().view(np.float16)
    )  # f32 bits carried in 2 f16 lanes, bitcast back on device
    # codebook gather, k-major: wraw[i, k*128+o] = cent[idx[o,i]][k], f16
    wraw = cent[idxT].transpose(0, 2, 1).reshape(P, 9 * P).astype(np.float16)
    wpackA = np.ascontiguousarray(np.concatenate(
        [scalesT.astype(np.float16), cutT.astype(np.float16), bias16,
         wraw[:, 0:3 * P]], axis=1))
    wpackB = np.ascontiguousarray(wraw[:, 3 * P:9 * P])

    base = {"wpackA": wpackA, "wpackB": wpackB}
    maps = []
    for c in range(N_CORES):
        m = dict(base)
        m["x"] = np.ascontiguousarray(x[IMGS * c:IMGS * (c + 1)])
        maps.append(m)
    return maps


def _get_nc():
    if "nc" not in _CACHE:
        _CACHE["nc"] = _build()
    return _CACHE["nc"]


def _run(inputs, trace=False):
    nc = _get_nc()
    in_maps = _make_in_maps(inputs)
    res = bass_utils.run_bass_kernel_spmd(
        nc, in_maps, core_ids=list(range(N_CORES)), trace=trace
    )
    out = np.concatenate([res.results[c]["out"] for c in range(N_CORES)], axis=0)
    return out, res


def kernel(**inputs) -> np.ndarray:
    out, _ = _run(inputs, trace=False)
    return out
